# revision 16
# baseline (speedup 1.0000x reference)
"""Trainium2 Bass kernel for nn_CBPoolMax2d.

Reference semantics: changeIndexes are flat spatial indices (y*W+x) of changed
input pixels; each maps to output pixel (y//2, x//2).  The output is the
persistent outputState with the 2x2-max-pooled value recomputed at every
changed output pixel (all channels).

Equivalent dense formulation used here:
    out = where(mask, maxpool2x2(input), outputState)
where mask[oy, ox] = any changeIndex maps to (oy, ox).  The mask is built on
host from the 128 KB index vector; all heavy data (input 256 MB, state 64 MB)
streams through the 8 NeuronCores, sharded over the channel dim (32 ch/core).

Per-core device kernel (measured fabric ceiling ~433 GB/s/core, so the whole
~52 MB/core stream is DMA-bound; everything is structured to keep both HWDGE
queues saturated with no load ever queued behind a dependent store):

  partitions = (channel, row-block): P = 32ch x 4rb = 128, where row-block rb
  owns the globally contiguous input rows [rb*128, rb*128+128) (output rows
  [rb*64, rb*64+64)).  Row tiles slice the free dim of every partition, so
  all DMA access patterns are 3-dim.

  scalar queue FIFO: [mask 2MB, full state 8.4MB, out writes as merges land]
  sync queue FIFO:   [input tiles only]
  per row tile:
    DMA input tile [128, r*512] f32         (sync HWDGE)
    hmax = max over col pairs               (DVE tensor_tensor, strided)
    vmax = max over row pairs               (DVE tensor_tensor, strided)
    copy_predicated(state_slice, mask, vmax)  -- in-place merge in SBUF
    DMA state_slice -> out DRAM             (scalar HWDGE)

The state for ALL tiles lives in one resident [128, 16384] f32 SBUF buffer
loaded by a single DMA at t=0, so no state load ever trails the pipeline and
the tail chain is just in->hmax->vmax->merge->out on the final small tile.
"""

import os
import numpy as np

C, H, W = 256, 512, 512
OH, OW = H // 2, W // 2
NCORES = 8
CPC = C // NCORES          # 32 channels per core

P = 128                    # SBUF partitions = (channel, row-block)
RB = P // CPC              # 4 row-blocks
HB = H // RB               # 128 input rows per row-block
OHB = OH // RB             # 64 output rows per row-block
# per-partition rows handled per tile (must be even); taper the tail so the
# last load->max->max->merge->store chain exposes less serial latency
TILE_R = [16] * 7 + [8, 4, 4]
assert sum(TILE_R) == HB
FREE_IN_MAX = max(TILE_R) * W                   # 8192
FREE_OUT_MAX = max(TILE_R) // 2 * OW            # 2048
FREE_ST = OHB * OW                              # 16384 output elems/partition

TRACE = os.environ.get("CBPOOL_TRACE", "0") == "1"
last_results = None

_cache = {}


def _build_nc():
    import concourse.bacc as bacc
    import concourse.tile as tile
    from concourse import bass, mybir

    f32 = mybir.dt.float32
    u8 = mybir.dt.uint8
    nc = bacc.Bacc("TRN2", target_bir_lowering=False, debug=False,
                   num_devices=NCORES)
    inp = nc.dram_tensor("inp", [CPC, H, W], f32, kind="ExternalInput")
    state = nc.dram_tensor("state", [CPC, OH, OW], f32, kind="ExternalInput")
    maskx = nc.dram_tensor("maskx", [RB, FREE_ST], u8, kind="ExternalInput")
    out = nc.dram_tensor("out", [CPC, OH, OW], f32, kind="ExternalOutput")

    with tile.TileContext(nc) as tc:
        with tc.tile_pool(name="pin", bufs=2) as pin, \
             tc.tile_pool(name="ph", bufs=1) as ph, \
             tc.tile_pool(name="pv", bufs=2) as pv, \
             tc.tile_pool(name="pm", bufs=1) as pm, \
             tc.tile_pool(name="pst", bufs=1) as pst:
            # scalar-queue FIFO: mask, then ALL state, then out writes.
            # Nothing on this queue ever waits behind a dependent store.
            # State is loaded per tile (not one monolithic DMA) so each
            # merge waits only on its own slice's completion semaphore.
            m_t = pm.tile([P, FREE_ST], u8)
            nc.scalar.dma_start(
                m_t,
                bass.AP(maskx, 0, [[0, CPC], [FREE_ST, RB], [1, FREE_ST]]))

            # per-tile slice offsets into the resident state buffer
            toff = [0]
            for r in TILE_R:
                toff.append(toff[-1] + (r // 2) * OW)

            st_t = pst.tile([P, FREE_ST], f32)
            for ti in range(len(TILE_R)):
                lo, hi = toff[ti], toff[ti + 1]
                nc.scalar.dma_start(
                    st_t[:, lo:hi],
                    bass.AP(state, lo,
                            [[OH * OW, CPC], [FREE_ST, RB], [1, hi - lo]]))

            def emit_merge(pend):
                soff, free_out, v_t = pend
                nc.vector.copy_predicated(out=st_t[:, soff:soff + free_out],
                                          mask=m_t[:, soff:soff + free_out],
                                          data=v_t[:, :free_out])
                nc.scalar.dma_start(
                    bass.AP(out, soff,
                            [[OH * OW, CPC], [FREE_ST, RB], [1, free_out]]),
                    st_t[:, soff:soff + free_out])

            # The merge for tile t is issued AFTER hmax/vmax of tile t+1, so
            # a merge waiting on its state slice never sits between h(t) and
            # h(t+1) in the DVE FIFO -- merge waits must not throttle input
            # buffer recycling (which gates the sync-queue input stream).
            pending = None
            trow0 = 0                 # within-block input row offset
            for ti, r in enumerate(TILE_R):
                free_in = r * W
                orpp = r // 2         # output rows per partition this tile
                free_out = orpp * OW
                soff = (trow0 // 2) * OW
                in_t = pin.tile([P, FREE_IN_MAX], f32)
                nc.sync.dma_start(
                    in_t[:, :free_in],
                    bass.AP(inp, trow0 * W,
                            [[H * W, CPC], [HB * W, RB], [1, free_in]]))

                # hmax over column pairs: [P, r, OW]
                h_t = ph.tile([P, max(TILE_R) * OW], f32)
                in_v = in_t[:, :free_in].rearrange("p (r x w) -> p r x w",
                                                   r=r, x=OW, w=2)
                h_v = h_t[:, :r * OW].rearrange("p (r x) -> p r x", r=r, x=OW)
                nc.vector.tensor_tensor(out=h_v, in0=in_v[:, :, :, 0],
                                        in1=in_v[:, :, :, 1],
                                        op=mybir.AluOpType.max)

                # vmax over row pairs: [P, orpp, OW]
                v_t = pv.tile([P, FREE_OUT_MAX], f32)
                h_vv = h_t[:, :r * OW].rearrange("p (r2 wr x) -> p r2 x wr",
                                                 r2=orpp, wr=2, x=OW)
                v_v = v_t[:, :free_out].rearrange("p (r2 x) -> p r2 x",
                                                  r2=orpp, x=OW)
                nc.vector.tensor_tensor(out=v_v, in0=h_vv[:, :, :, 0],
                                        in1=h_vv[:, :, :, 1],
                                        op=mybir.AluOpType.max)

                if pending is not None:
                    emit_merge(pending)
                pending = (soff, free_out, v_t)
                trow0 += r
            emit_merge(pending)

    nc.compile()
    return nc


def _get_nc():
    if "nc" not in _cache:
        _cache["nc"] = _build_nc()
    return _cache["nc"]


def _build_maskx(changeIndexes):
    """Expanded u8 mask in per-partition layout: maskx[rb] = output rows
    [rb*64, rb*64+64) flattened.  64 KB of index-derived metadata, replicated
    over channels by a stride-0 DMA on device."""
    ci = np.asarray(changeIndexes).astype(np.int64)
    oy = (ci // W) // 2
    ox = (ci % W) // 2
    mask = np.zeros((OH, OW), dtype=np.uint8)
    mask[oy, ox] = 1
    return np.ascontiguousarray(mask.reshape(RB, FREE_ST))


def kernel(input, outputState, changeIndexes):
    global last_results
    from concourse.bass_utils import run_bass_kernel_spmd

    nc = _get_nc()

    inp = np.ascontiguousarray(
        np.asarray(input, dtype=np.float32).reshape(C, H, W))
    state = np.ascontiguousarray(
        np.asarray(outputState, dtype=np.float32).reshape(C, OH, OW))
    maskx = _build_maskx(changeIndexes)

    in_maps = [
        {
            "inp": inp[i * CPC:(i + 1) * CPC],
            "state": state[i * CPC:(i + 1) * CPC],
            "maskx": maskx,
        }
        for i in range(NCORES)
    ]
    res = run_bass_kernel_spmd(nc, in_maps, core_ids=list(range(NCORES)),
                               trace=TRACE)
    last_results = res
    out = np.concatenate([res.results[i]["out"] for i in range(NCORES)],
                         axis=0)
    return out.reshape(1, C, OH, OW).astype(np.float32, copy=False)


# revision 18
# speedup vs baseline: 1.0136x; 1.0136x over previous
"""Trainium2 Bass kernel for nn_CBPoolMax2d.

Reference semantics: changeIndexes are flat spatial indices (y*W+x) of changed
input pixels; each maps to output pixel (y//2, x//2).  The output is the
persistent outputState with the 2x2-max-pooled value recomputed at every
changed output pixel (all channels).

Equivalent dense formulation used here:
    out = where(mask, maxpool2x2(input), outputState)
where mask[oy, ox] = any changeIndex maps to (oy, ox).  The mask is built on
host from the 128 KB index vector; all heavy data (input 256 MB, state 64 MB)
streams through the 8 NeuronCores, sharded over the channel dim (32 ch/core).

Per-core device kernel (measured fabric ceiling ~433 GB/s/core, so the whole
~52 MB/core stream is DMA-bound; everything is structured to keep both HWDGE
queues saturated with no load ever queued behind a dependent store):

  partitions = (channel, row-block): P = 32ch x 4rb = 128, where row-block rb
  owns the globally contiguous input rows [rb*128, rb*128+128) (output rows
  [rb*64, rb*64+64)).  Row tiles slice the free dim of every partition, so
  all DMA access patterns are 3-dim.

  scalar queue FIFO: [mask 2MB, full state 8.4MB, out writes as merges land]
  sync queue FIFO:   [input tiles only]
  per row tile:
    DMA input tile [128, r*512] f32         (sync HWDGE)
    hmax = max over col pairs               (DVE tensor_tensor, strided)
    vmax = max over row pairs               (DVE tensor_tensor, strided)
    copy_predicated(state_slice, mask, vmax)  -- in-place merge in SBUF
    DMA state_slice -> out DRAM             (scalar HWDGE)

The state for ALL tiles lives in one resident [128, 16384] f32 SBUF buffer
loaded by a single DMA at t=0, so no state load ever trails the pipeline and
the tail chain is just in->hmax->vmax->merge->out on the final small tile.
"""

import os
import numpy as np

C, H, W = 256, 512, 512
OH, OW = H // 2, W // 2
NCORES = 8
CPC = C // NCORES          # 32 channels per core

P = 128                    # SBUF partitions = (channel, row-block)
RB = P // CPC              # 4 row-blocks
HB = H // RB               # 128 input rows per row-block
OHB = OH // RB             # 64 output rows per row-block
# per-partition rows handled per tile (must be even); taper the tail so the
# last load->max->max->merge->store chain exposes less serial latency
TILE_R = [16] * 7 + [8, 4, 4]
assert sum(TILE_R) == HB
FREE_IN_MAX = max(TILE_R) * W                   # 8192
FREE_OUT_MAX = max(TILE_R) // 2 * OW            # 2048
FREE_ST = OHB * OW                              # 16384 output elems/partition

TRACE = os.environ.get("CBPOOL_TRACE", "0") == "1"
last_results = None

_cache = {}


def _build_nc():
    import concourse.bacc as bacc
    import concourse.tile as tile
    from concourse import bass, mybir

    f32 = mybir.dt.float32
    u8 = mybir.dt.uint8
    nc = bacc.Bacc("TRN2", target_bir_lowering=False, debug=False,
                   num_devices=NCORES)
    inp = nc.dram_tensor("inp", [CPC, H, W], f32, kind="ExternalInput")
    state = nc.dram_tensor("state", [CPC, OH, OW], f32, kind="ExternalInput")
    maskx = nc.dram_tensor("maskx", [RB, FREE_ST], u8, kind="ExternalInput")
    out = nc.dram_tensor("out", [CPC, OH, OW], f32, kind="ExternalOutput")

    with tile.TileContext(nc) as tc:
        with tc.tile_pool(name="pin", bufs=3) as pin, \
             tc.tile_pool(name="ph", bufs=1) as ph, \
             tc.tile_pool(name="pv", bufs=2, space="PSUM") as pv, \
             tc.tile_pool(name="pm", bufs=1) as pm, \
             tc.tile_pool(name="pst", bufs=1) as pst:
            # scalar-queue FIFO: mask, then ALL state, then out writes.
            # Nothing on this queue ever waits behind a dependent store.
            # State is loaded per tile (not one monolithic DMA) so each
            # merge waits only on its own slice's completion semaphore.
            m_t = pm.tile([P, FREE_ST], u8)
            nc.scalar.dma_start(
                m_t,
                bass.AP(maskx, 0, [[0, CPC], [FREE_ST, RB], [1, FREE_ST]]))

            # per-tile slice offsets into the resident state buffer
            toff = [0]
            for r in TILE_R:
                toff.append(toff[-1] + (r // 2) * OW)

            st_t = pst.tile([P, FREE_ST], f32)
            for ti in range(len(TILE_R)):
                lo, hi = toff[ti], toff[ti + 1]
                nc.scalar.dma_start(
                    st_t[:, lo:hi],
                    bass.AP(state, lo,
                            [[OH * OW, CPC], [FREE_ST, RB], [1, hi - lo]]))

            def emit_merge(pend):
                soff, free_out, v_t = pend
                nc.vector.copy_predicated(out=st_t[:, soff:soff + free_out],
                                          mask=m_t[:, soff:soff + free_out],
                                          data=v_t[:, :free_out])
                nc.scalar.dma_start(
                    bass.AP(out, soff,
                            [[OH * OW, CPC], [FREE_ST, RB], [1, free_out]]),
                    st_t[:, soff:soff + free_out])

            # The merge for tile t is issued AFTER hmax/vmax of tile t+1, so
            # a merge waiting on its state slice never sits between h(t) and
            # h(t+1) in the DVE FIFO -- merge waits must not throttle input
            # buffer recycling (which gates the sync-queue input stream).
            pending = None
            trow0 = 0                 # within-block input row offset
            for ti, r in enumerate(TILE_R):
                free_in = r * W
                orpp = r // 2         # output rows per partition this tile
                free_out = orpp * OW
                soff = (trow0 // 2) * OW
                in_t = pin.tile([P, FREE_IN_MAX], f32)
                nc.sync.dma_start(
                    in_t[:, :free_in],
                    bass.AP(inp, trow0 * W,
                            [[H * W, CPC], [HB * W, RB], [1, free_in]]))

                # hmax over column pairs: [P, r, OW]
                h_t = ph.tile([P, max(TILE_R) * OW], f32)
                in_v = in_t[:, :free_in].rearrange("p (r x w) -> p r x w",
                                                   r=r, x=OW, w=2)
                h_v = h_t[:, :r * OW].rearrange("p (r x) -> p r x", r=r, x=OW)
                nc.vector.tensor_tensor(out=h_v, in0=in_v[:, :, :, 0],
                                        in1=in_v[:, :, :, 1],
                                        op=mybir.AluOpType.max)

                # vmax over row pairs: [P, orpp, OW]
                v_t = pv.tile([P, FREE_OUT_MAX], f32)
                h_vv = h_t[:, :r * OW].rearrange("p (r2 wr x) -> p r2 x wr",
                                                 r2=orpp, wr=2, x=OW)
                v_v = v_t[:, :free_out].rearrange("p (r2 x) -> p r2 x",
                                                  r2=orpp, x=OW)
                nc.vector.tensor_tensor(out=v_v, in0=h_vv[:, :, :, 0],
                                        in1=h_vv[:, :, :, 1],
                                        op=mybir.AluOpType.max)

                if pending is not None:
                    emit_merge(pending)
                pending = (soff, free_out, v_t)
                trow0 += r
            emit_merge(pending)

    nc.compile()
    return nc


def _get_nc():
    if "nc" not in _cache:
        _cache["nc"] = _build_nc()
    return _cache["nc"]


def _build_maskx(changeIndexes):
    """Expanded u8 mask in per-partition layout: maskx[rb] = output rows
    [rb*64, rb*64+64) flattened.  64 KB of index-derived metadata, replicated
    over channels by a stride-0 DMA on device."""
    ci = np.asarray(changeIndexes).astype(np.int64)
    oy = (ci // W) // 2
    ox = (ci % W) // 2
    mask = np.zeros((OH, OW), dtype=np.uint8)
    mask[oy, ox] = 1
    return np.ascontiguousarray(mask.reshape(RB, FREE_ST))


def kernel(input, outputState, changeIndexes):
    global last_results
    from concourse.bass_utils import run_bass_kernel_spmd

    nc = _get_nc()

    inp = np.ascontiguousarray(
        np.asarray(input, dtype=np.float32).reshape(C, H, W))
    state = np.ascontiguousarray(
        np.asarray(outputState, dtype=np.float32).reshape(C, OH, OW))
    maskx = _build_maskx(changeIndexes)

    in_maps = [
        {
            "inp": inp[i * CPC:(i + 1) * CPC],
            "state": state[i * CPC:(i + 1) * CPC],
            "maskx": maskx,
        }
        for i in range(NCORES)
    ]
    res = run_bass_kernel_spmd(nc, in_maps, core_ids=list(range(NCORES)),
                               trace=TRACE)
    last_results = res
    out = np.concatenate([res.results[i]["out"] for i in range(NCORES)],
                         axis=0)
    return out.reshape(1, C, OH, OW).astype(np.float32, copy=False)


# revision 20
# speedup vs baseline: 1.2252x; 1.2088x over previous
"""Trainium2 Bass kernel for nn_CBPoolMax2d.

Reference semantics: changeIndexes are flat spatial indices (y*W+x) of changed
input pixels; each maps to output pixel (y//2, x//2).  The output is the
persistent outputState with the 2x2-max-pooled value recomputed at every
changed output pixel (all channels).

Equivalent dense formulation used here:
    out = where(mask, maxpool2x2(input), outputState)
where mask[oy, ox] = any changeIndex maps to (oy, ox).  The mask is built on
host from the 128 KB index vector; all heavy data (input 256 MB, state 64 MB)
streams through the 8 NeuronCores, sharded over the channel dim (32 ch/core).

Per-core device kernel (measured fabric ceiling ~433 GB/s/core, so the whole
~52 MB/core stream is DMA-bound; everything is structured to keep both HWDGE
queues saturated with no load ever queued behind a dependent store):

  partitions = (channel, row-block): P = 32ch x 4rb = 128, where row-block rb
  owns the globally contiguous input rows [rb*128, rb*128+128) (output rows
  [rb*64, rb*64+64)).  Row tiles slice the free dim of every partition, so
  all DMA access patterns are 3-dim.

  scalar queue FIFO: [mask 2MB, full state 8.4MB, out writes as merges land]
  sync queue FIFO:   [input tiles only]
  per row tile:
    DMA input tile [128, r*512] f32         (sync HWDGE)
    hmax = max over col pairs               (DVE tensor_tensor, strided)
    vmax = max over row pairs               (DVE tensor_tensor, strided)
    copy_predicated(state_slice, mask, vmax)  -- in-place merge in SBUF
    DMA state_slice -> out DRAM             (scalar HWDGE)

The state for ALL tiles lives in one resident [128, 16384] f32 SBUF buffer
loaded by a single DMA at t=0, so no state load ever trails the pipeline and
the tail chain is just in->hmax->vmax->merge->out on the final small tile.
"""

import os
import numpy as np

C, H, W = 256, 512, 512
OH, OW = H // 2, W // 2
NCORES = 8
CPC = C // NCORES          # 32 channels per core

P = 128                    # SBUF partitions = (channel, row-block)
RB = P // CPC              # 4 row-blocks
HB = H // RB               # 128 input rows per row-block
OHB = OH // RB             # 64 output rows per row-block
# per-partition rows handled per tile (must be even); taper the tail so the
# last load->max->max->merge->store chain exposes less serial latency
TILE_R = [16] * 7 + [8, 4, 4]
assert sum(TILE_R) == HB
FREE_IN_MAX = max(TILE_R) * W                   # 8192
FREE_OUT_MAX = max(TILE_R) // 2 * OW            # 2048
FREE_ST = OHB * OW                              # 16384 output elems/partition

TRACE = os.environ.get("CBPOOL_TRACE", "0") == "1"
last_results = None

_cache = {}


def _build_nc():
    import concourse.bacc as bacc
    import concourse.tile as tile
    from concourse import bass, mybir

    f32 = mybir.dt.float32
    u8 = mybir.dt.uint8
    nc = bacc.Bacc("TRN2", target_bir_lowering=False, debug=False,
                   num_devices=NCORES)
    inp = nc.dram_tensor("inp", [CPC, H, W], f32, kind="ExternalInput")
    state = nc.dram_tensor("state", [CPC, OH, OW], f32, kind="ExternalInput")
    maskx = nc.dram_tensor("maskx", [RB, FREE_ST], u8, kind="ExternalInput")
    out = nc.dram_tensor("out", [CPC, OH, OW], f32, kind="ExternalOutput")

    with tile.TileContext(nc) as tc:
        with tc.tile_pool(name="pin", bufs=3) as pin, \
             tc.tile_pool(name="ph", bufs=1) as ph, \
             tc.tile_pool(name="pv", bufs=2, space="PSUM") as pv, \
             tc.tile_pool(name="pm", bufs=1) as pm, \
             tc.tile_pool(name="pst", bufs=1) as pst:
            # scalar-queue FIFO: mask, then ALL state, then out writes.
            # Nothing on this queue ever waits behind a dependent store.
            # State is loaded per tile (not one monolithic DMA) so each
            # merge waits only on its own slice's completion semaphore.
            m_t = pm.tile([P, FREE_ST], u8)
            nc.scalar.dma_start(
                m_t,
                bass.AP(maskx, 0, [[0, CPC], [FREE_ST, RB], [1, FREE_ST]]))

            # per-tile slice offsets into the resident state buffer
            toff = [0]
            for r in TILE_R:
                toff.append(toff[-1] + (r // 2) * OW)

            # State preload in 4 tile-aligned chunks.  DMA-completion
            # semaphore lanes are assigned round-robin over dma_start
            # CREATION order (8 lanes), so the creation order below is
            # engineered: [mask, st x4, in x10, out x10].  Inputs then wrap
            # onto state chunks that complete before the input's own buffer
            # dependency clears, and outs wrap onto long-finished inputs --
            # no dispatch ever blocks on a slow lane predecessor.
            st_t = pst.tile([P, FREE_ST], f32)
            for ta, tb in [(0, 2), (2, 4), (4, 6), (6, len(TILE_R))]:
                lo, hi = toff[ta], toff[tb]
                nc.scalar.dma_start(
                    st_t[:, lo:hi],
                    bass.AP(state, lo,
                            [[OH * OW, CPC], [FREE_ST, RB], [1, hi - lo]]))

            # create ALL input-tile DMAs up front (sync queue FIFO; the
            # engine stalls on each tile's buffer-free wait, which is the
            # intended pacing)
            in_tiles = []
            trow0 = 0
            for r in TILE_R:
                in_t = pin.tile([P, FREE_IN_MAX], f32)
                nc.sync.dma_start(
                    in_t[:, :r * W],
                    bass.AP(inp, trow0 * W,
                            [[H * W, CPC], [HB * W, RB], [1, r * W]]))
                in_tiles.append(in_t)
                trow0 += r

            def emit_merge(pend):
                soff, free_out, v_t = pend
                nc.vector.copy_predicated(out=st_t[:, soff:soff + free_out],
                                          mask=m_t[:, soff:soff + free_out],
                                          data=v_t[:, :free_out])
                nc.scalar.dma_start(
                    bass.AP(out, soff,
                            [[OH * OW, CPC], [FREE_ST, RB], [1, free_out]]),
                    st_t[:, soff:soff + free_out])

            # The merge for tile t is issued AFTER hmax/vmax of tile t+1, so
            # a merge waiting on its state slice never sits between h(t) and
            # h(t+1) in the DVE FIFO -- merge waits must not throttle input
            # buffer recycling (which gates the sync-queue input stream).
            pending = None
            trow0 = 0                 # within-block input row offset
            for ti, r in enumerate(TILE_R):
                free_in = r * W
                orpp = r // 2         # output rows per partition this tile
                free_out = orpp * OW
                soff = (trow0 // 2) * OW
                in_t = in_tiles[ti]

                # hmax over column pairs: [P, r, OW]
                h_t = ph.tile([P, max(TILE_R) * OW], f32)
                in_v = in_t[:, :free_in].rearrange("p (r x w) -> p r x w",
                                                   r=r, x=OW, w=2)
                h_v = h_t[:, :r * OW].rearrange("p (r x) -> p r x", r=r, x=OW)
                nc.vector.tensor_tensor(out=h_v, in0=in_v[:, :, :, 0],
                                        in1=in_v[:, :, :, 1],
                                        op=mybir.AluOpType.max)

                # vmax over row pairs: [P, orpp, OW]
                v_t = pv.tile([P, FREE_OUT_MAX], f32)
                h_vv = h_t[:, :r * OW].rearrange("p (r2 wr x) -> p r2 x wr",
                                                 r2=orpp, wr=2, x=OW)
                v_v = v_t[:, :free_out].rearrange("p (r2 x) -> p r2 x",
                                                  r2=orpp, x=OW)
                nc.vector.tensor_tensor(out=v_v, in0=h_vv[:, :, :, 0],
                                        in1=h_vv[:, :, :, 1],
                                        op=mybir.AluOpType.max)

                if pending is not None:
                    emit_merge(pending)
                pending = (soff, free_out, v_t)
                trow0 += r
            emit_merge(pending)

    nc.compile()
    return nc


def _get_nc():
    if "nc" not in _cache:
        _cache["nc"] = _build_nc()
    return _cache["nc"]


def _build_maskx(changeIndexes):
    """Expanded u8 mask in per-partition layout: maskx[rb] = output rows
    [rb*64, rb*64+64) flattened.  64 KB of index-derived metadata, replicated
    over channels by a stride-0 DMA on device."""
    ci = np.asarray(changeIndexes).astype(np.int64)
    oy = (ci // W) // 2
    ox = (ci % W) // 2
    mask = np.zeros((OH, OW), dtype=np.uint8)
    mask[oy, ox] = 1
    return np.ascontiguousarray(mask.reshape(RB, FREE_ST))


def kernel(input, outputState, changeIndexes):
    global last_results
    from concourse.bass_utils import run_bass_kernel_spmd

    nc = _get_nc()

    inp = np.ascontiguousarray(
        np.asarray(input, dtype=np.float32).reshape(C, H, W))
    state = np.ascontiguousarray(
        np.asarray(outputState, dtype=np.float32).reshape(C, OH, OW))
    maskx = _build_maskx(changeIndexes)

    in_maps = [
        {
            "inp": inp[i * CPC:(i + 1) * CPC],
            "state": state[i * CPC:(i + 1) * CPC],
            "maskx": maskx,
        }
        for i in range(NCORES)
    ]
    res = run_bass_kernel_spmd(nc, in_maps, core_ids=list(range(NCORES)),
                               trace=TRACE)
    last_results = res
    out = np.concatenate([res.results[i]["out"] for i in range(NCORES)],
                         axis=0)
    return out.reshape(1, C, OH, OW).astype(np.float32, copy=False)
